# revision 2
# baseline (speedup 1.0000x reference)
"""Trainium2 Bass kernel for BertSelfAttention + LoRA (bs=4, seq=2048, hidden=1024, 16 heads).

Sharding: 8 cores = 4 batches x 2 head-groups. Each core handles one batch and 8
heads (512 of the 1024 hidden output dims). LoRA is folded into the weights on
the host (W_eff = W + scaling * B @ A  — algebraically identical).

Per-core device kernel (all matmuls bf16, accumulation fp32 in PSUM):
  x [2048,1024] f32  --SWDGE cast DMA-->  x16 (DRAM bf16)  --xbar transpose-->
  xT [1024,2048] in SBUF (8 tiles of [128,2048]).
  W slices likewise -> wT [1024,512] per projection.
  QK^T projections computed transposed:  qT/kT [d'=512, tok] (bias folded into
  the PSUM->SBUF cast via per-partition tensor_scalar_add).
  V computed in natural [tok, d'] layout (bias via a K=1 ones-row matmul pass),
  assembled into V' tiles [128, 8*65] with a ones column per head.
  Attention per head, fully transposed:
    scoresT[k,q] = K^T(d,k).T @ Q^T(d,q)      (PSUM [128,1024])
    expT = exp(scoresT/8 + mask[k])           (ACT, mask = per-partition bias)
    outT[d',q] += V'[k,d'].T @ expT[k,q]      (PSUM [65,512]; row 64 = denom)
  Host divides by the denominator row and transposes during the gather.
"""

import numpy as np

BS = 4
SEQ = 2048
HID = 1024
HEADS = 16
HD = 64
RANK = 16
LORA_SCALING = 1.0 / RANK

N_CORES = 8
NH = 8          # heads per core
DP = 512        # output dims per core (NH * HD)
P = 128
NT = SEQ // P   # 16 token tiles
NHB = HID // P  # 8 hidden blocks
NG = DP // P    # 4 d' groups (2 heads each)
VW = HD + 1     # 65: V columns + ones column

_CACHE = {}


def _build():
    import concourse.bass as bass
    import concourse.tile as tile
    from concourse import bacc, mybir

    f32 = mybir.dt.float32
    bf16 = mybir.dt.bfloat16
    Exp = mybir.ActivationFunctionType.Exp

    nc = bacc.Bacc("TRN2", target_bir_lowering=False, debug=False,
                   num_devices=N_CORES)

    x = nc.dram_tensor("x", [SEQ, HID], f32, kind="ExternalInput").ap()
    w_in = [nc.dram_tensor(f"w{n}", [DP, HID], f32, kind="ExternalInput").ap()
            for n in "qkv"]
    # host pre-rearranged: bias_qk[i] is [128, NG] (partition = d' within group)
    bias_qk = [nc.dram_tensor(f"b{n}", [P, NG], f32, kind="ExternalInput").ap()
               for n in "qk"]
    bv = nc.dram_tensor("bv", [1, DP], f32, kind="ExternalInput").ap()
    # host pre-rearranged mask: [128, NT] (partition = k within tile)
    mask = nc.dram_tensor("mask", [P, NT], f32, kind="ExternalInput").ap()
    out = nc.dram_tensor("out", [NH, VW, SEQ], f32, kind="ExternalOutput").ap()

    x16 = nc.dram_tensor("x16", [SEQ, HID], bf16).ap()
    w16 = nc.dram_tensor("w16", [3, DP, HID], bf16).ap()

    with tile.TileContext(nc) as tc:
        with (
            tc.tile_pool(name="consts", bufs=1) as cpool,
            tc.tile_pool(name="xT", bufs=1) as xT_pool,
            tc.tile_pool(name="wT", bufs=1) as wT_pool,
            tc.tile_pool(name="qkT", bufs=1) as qkT_pool,
            tc.tile_pool(name="vp", bufs=1) as vp_pool,
            tc.tile_pool(name="expp", bufs=3) as exp_pool,
            tc.tile_pool(name="outp", bufs=3) as out_pool,
            tc.tile_pool(name="ps_proj", bufs=2, space="PSUM") as ps_proj,
            tc.tile_pool(name="ps_sc", bufs=2, space="PSUM") as ps_sc,
            tc.tile_pool(name="ps_pv", bufs=2, space="PSUM") as ps_pv,
        ):
            # ---- constants ----
            mask_t = cpool.tile([P, NT], f32, tag="mask", name="mask_t")
            nc.sync.dma_start(mask_t[:], mask[:, :])
            bias_t = []
            for i in range(2):
                bt = cpool.tile([P, NG], f32, tag=f"bias{i}", name=f"bias{i}")
                nc.sync.dma_start(bt[:], bias_qk[i][:, :])
                bias_t.append(bt)
            bv_f = cpool.tile([1, DP], f32, tag="bvf", name="bvf")
            nc.sync.dma_start(bv_f[:], bv[:, :])
            bv16 = cpool.tile([1, DP], bf16, tag="bv16", name="bv16")
            nc.vector.tensor_copy(bv16[:], bv_f[:])
            ones_row = cpool.tile([1, P], bf16, tag="ones", name="ones_row")
            nc.gpsimd.memset(ones_row[:], 1.0)

            # ---- cast inputs to bf16 in DRAM (SWDGE cast DMA) ----
            for rc in range(4):
                r = slice(rc * 512, (rc + 1) * 512)
                nc.gpsimd.dma_start(x16[r, :], x[r, :])
            for w in range(3):
                nc.gpsimd.dma_start(w16[w], w_in[w][:, :])

            # ---- xbar transposes into SBUF ----
            wT = [[wT_pool.tile([P, DP], bf16, tag=f"wT{w}_{c}", name=f"wT{w}_{c}")
                   for c in range(NHB)] for w in range(3)]
            for w in range(3):
                for c in range(NHB):
                    nc.sync.dma_start(wT[w][c][:],
                                      w16[w][:, c * P:(c + 1) * P],
                                      transpose=True)
            xT = [xT_pool.tile([P, SEQ], bf16, tag=f"xT{c}", name=f"xT{c}") for c in range(NHB)]
            for c in range(NHB):
                for rc in range(4):
                    r = slice(rc * 512, (rc + 1) * 512)
                    nc.sync.dma_start(xT[c][:, r],
                                      x16[r, c * P:(c + 1) * P],
                                      transpose=True)

            # ---- V projection (natural layout) + V' assembly ----
            vp = vp_pool.tile([P, NT * NH * VW], bf16, tag="vp", name="vp")
            nc.gpsimd.memset(vp[:], 1.0)
            for tt in range(NT):
                ps = ps_proj.tile([P, DP], f32, tag="proj", name="ps_v")
                for p in range(NHB):
                    nc.tensor.matmul(ps[:],
                                     xT[p][:, tt * P:(tt + 1) * P],
                                     wT[2][p][:],
                                     start=(p == 0), stop=False)
                nc.tensor.matmul(ps[:], ones_row[:], bv16[:],
                                 start=False, stop=True)
                base = tt * NH * VW
                dst = vp[:, base:base + NH * VW]
                dst = dst.rearrange("p (h c) -> p h c", c=VW)[:, :, 0:HD]
                nc.vector.tensor_copy(dst, ps[:].rearrange("p (h c) -> p h c",
                                                           c=HD))

            # ---- Q^T / K^T projections (transposed layout) ----
            qkT = [[qkT_pool.tile([P, SEQ], bf16, tag=f"qkT{w}_{g}", name=f"qkT{w}_{g}")
                    for g in range(NG)] for w in range(2)]
            for g in range(NG):
                gs = slice(g * P, (g + 1) * P)
                for w in range(2):
                    for tp in range(2):
                        pss = [ps_proj.tile([P, 512], f32, tag="proj", name="ps_qk")
                               for _ in range(2)]
                        for p in range(NHB):
                            for half in range(2):
                                tch = tp * 2 + half
                                nc.tensor.matmul(
                                    pss[half][:],
                                    wT[w][p][:, gs],
                                    xT[p][:, tch * 512:(tch + 1) * 512],
                                    start=(p == 0), stop=(p == NHB - 1))
                        for half in range(2):
                            tch = tp * 2 + half
                            nc.vector.tensor_scalar_add(
                                qkT[w][g][:, tch * 512:(tch + 1) * 512],
                                pss[half][:],
                                bias_t[w][:, g:g + 1])

                # ---- attention for the two heads of this group ----
                for h2 in range(2):
                    h = g * 2 + h2
                    hp = slice(h2 * HD, (h2 + 1) * HD)
                    for qb in range(2):
                        pvt = None
                        for kt in range(NT):
                            sc = ps_sc.tile([P, 1024], f32, tag="sc", name="sc")
                            for qh in range(2):
                                q0 = qb * 1024 + qh * 512
                                nc.tensor.matmul(
                                    sc[:, qh * 512:(qh + 1) * 512],
                                    qkT[1][g][hp, kt * P:(kt + 1) * P],
                                    qkT[0][g][hp, q0:q0 + 512],
                                    start=True, stop=True)
                            et = exp_pool.tile([P, 1024], bf16, tag="exp", name="et")
                            nc.scalar.activation(et[:], sc[:], Exp,
                                                 bias=mask_t[:, kt:kt + 1],
                                                 scale=0.125)
                            if kt == 0:
                                pvt = [ps_pv.tile([VW, 512], f32, tag="pv", name="pv")
                                       for _ in range(2)]
                            vbase = kt * NH * VW + h * VW
                            for qc in range(2):
                                nc.tensor.matmul(
                                    pvt[qc][:],
                                    vp[:, vbase:vbase + VW],
                                    et[:, qc * 512:(qc + 1) * 512],
                                    start=(kt == 0), stop=(kt == NT - 1))
                        for qc in range(2):
                            ot = out_pool.tile([VW, 512], f32, tag="ot", name="ot")
                            nc.vector.tensor_copy(ot[:], pvt[qc][:])
                            q0 = qb * 1024 + qc * 512
                            nc.sync.dma_start(out[h][:, q0:q0 + 512], ot[:])

    nc.compile()
    return nc


def _get_nc():
    if "nc" not in _CACHE:
        _CACHE["nc"] = _build()
    return _CACHE["nc"]


def kernel(hidden_states, attention_mask, Wq, bq, Aq, Bq, Wk, bk, Ak, Bk,
           Wv, bv, Av, Bv):
    from concourse import bass_utils
    import os

    nc = _get_nc()

    hs = np.ascontiguousarray(np.asarray(hidden_states, dtype=np.float32))
    am = np.asarray(attention_mask, dtype=np.float32)
    weff = {}
    for n, W, A, B in (("q", Wq, Aq, Bq), ("k", Wk, Ak, Bk), ("v", Wv, Av, Bv)):
        W = np.asarray(W, dtype=np.float32)
        A = np.asarray(A, dtype=np.float32)
        B = np.asarray(B, dtype=np.float32)
        weff[n] = W + LORA_SCALING * (B @ A)
    biases = {"q": np.asarray(bq, np.float32), "k": np.asarray(bk, np.float32),
              "v": np.asarray(bv, np.float32)}

    in_maps = []
    for c in range(N_CORES):
        b, hg = divmod(c, 2)
        rows = slice(hg * DP, (hg + 1) * DP)
        m = {
            "x": hs[b],
            "mask": np.ascontiguousarray(am[b, 0, 0].reshape(NT, P).T),
            "bv": biases["v"][rows].reshape(1, DP),
        }
        for n in ("q", "k", "v"):
            m[f"w{n}"] = np.ascontiguousarray(weff[n][rows])
        for n in ("q", "k"):
            m[f"b{n}"] = np.ascontiguousarray(
                biases[n][rows].reshape(NG, P).T)
        in_maps.append(m)

    trace = bool(int(os.environ.get("BASS_KERNEL_TRACE", "0")))
    res = bass_utils.run_bass_kernel_spmd(nc, in_maps,
                                          core_ids=list(range(N_CORES)),
                                          trace=trace)
    _CACHE["last_results"] = res

    output = np.empty((BS, SEQ, HID), dtype=np.float32)
    for c in range(N_CORES):
        b, hg = divmod(c, 2)
        r = res.results[c]["out"]                      # [NH, 65, SEQ]
        o = r[:, :HD, :] / r[:, HD:HD + 1, :]          # [NH, 64, SEQ]
        output[b, :, hg * DP:(hg + 1) * DP] = (
            o.transpose(2, 0, 1).reshape(SEQ, DP))
    return output


# revision 3
# speedup vs baseline: 1.0468x; 1.0468x over previous
"""Trainium2 Bass kernel for BertSelfAttention + LoRA (bs=4, seq=2048, hidden=1024, 16 heads).

Sharding: 8 cores = 4 batches x 2 head-groups. Each core handles one batch and 8
heads (512 of the 1024 hidden output dims). LoRA is folded into the weights on
the host (W_eff = W + scaling * B @ A  — algebraically identical), and x / W_eff
are pre-cast to bf16 on the host.

Per-core device kernel (all matmuls bf16, accumulation fp32 in PSUM):
  x16 [2048,1024] bf16  --xbar transpose DMA-->  xT [1024,2048] in SBUF
  W slices likewise -> wT [1024,512] per projection.
  QK^T projections computed transposed:  qT/kT [d'=512, tok] (bias folded into
  the PSUM->SBUF cast via per-partition tensor_scalar_add).
  V computed in natural [tok, d'] layout (bias via a K=1 ones-row matmul pass),
  assembled into V' tiles [128, 8*65] with a ones column per head.
  Attention per head PAIR (row-packed: head0 on array rows 0-63, head1 on rows
  64-127, concurrent via PE row groups), fully transposed:
    scoresT[k,q] = K^T(d,k).T @ Q^T(d,q)      (PSUM [128,1024])
    expT = exp(scoresT/8 + mask[k])           (ACT, mask = per-partition bias)
    outT[d',q] += V'[k,d'].T @ expT[k,q]      (PSUM [65,512]; row 64 = denom)
  head0's PV runs inline; head1's exp tiles persist and its PV runs as a second
  pass, so only 2 PV PSUM banks are live at a time (total 8 banks exactly).
  Host divides by the denominator row and transposes during the gather.
"""

import numpy as np

BS = 4
SEQ = 2048
HID = 1024
HEADS = 16
HD = 64
RANK = 16
LORA_SCALING = 1.0 / RANK

N_CORES = 8
NH = 8          # heads per core
DP = 512        # output dims per core (NH * HD)
P = 128
NT = SEQ // P   # 16 token tiles
NHB = HID // P  # 8 hidden blocks
NG = DP // P    # 4 d' groups (2 heads each)
VW = HD + 1     # 65: V columns + ones column

_CACHE = {}


def _build():
    import concourse.bass as bass
    import concourse.tile as tile
    from concourse import bacc, mybir

    f32 = mybir.dt.float32
    bf16 = mybir.dt.bfloat16
    Exp = mybir.ActivationFunctionType.Exp

    nc = bacc.Bacc("TRN2", target_bir_lowering=False, debug=False,
                   num_devices=N_CORES)

    x16 = nc.dram_tensor("x16", [SEQ, HID], bf16, kind="ExternalInput").ap()
    w_in = [nc.dram_tensor(f"w{n}", [DP, HID], bf16, kind="ExternalInput").ap()
            for n in "qkv"]
    # host pre-rearranged: bias_qk[i] is [128, NG] (partition = d' within group)
    bias_qk = [nc.dram_tensor(f"b{n}", [P, NG], f32, kind="ExternalInput").ap()
               for n in "qk"]
    bv = nc.dram_tensor("bv", [1, DP], f32, kind="ExternalInput").ap()
    # host pre-rearranged mask: [128, NT] (partition = k within tile)
    mask = nc.dram_tensor("mask", [P, NT], f32, kind="ExternalInput").ap()
    out = nc.dram_tensor("out", [NH, VW, SEQ], f32, kind="ExternalOutput").ap()

    with tile.TileContext(nc) as tc:
        with (
            tc.tile_pool(name="consts", bufs=1) as cpool,
            tc.tile_pool(name="xT", bufs=1) as xT_pool,
            tc.tile_pool(name="wT", bufs=1) as wT_pool,
            tc.tile_pool(name="qkT", bufs=1) as qkT_pool,
            tc.tile_pool(name="vp", bufs=1) as vp_pool,
            tc.tile_pool(name="expp", bufs=3) as exp_pool,
            tc.tile_pool(name="exp1", bufs=16) as exp1_pool,
            tc.tile_pool(name="outp", bufs=3) as out_pool,
            tc.tile_pool(name="ps_proj", bufs=2, space="PSUM") as ps_proj,
            tc.tile_pool(name="ps_sc", bufs=2, space="PSUM") as ps_sc,
            tc.tile_pool(name="ps_pv", bufs=2, space="PSUM") as ps_pv,
        ):
            # ---- constants ----
            mask_t = cpool.tile([P, NT], f32, tag="mask", name="mask_t")
            nc.sync.dma_start(mask_t[:], mask[:, :])
            bias_t = []
            for i in range(2):
                bt = cpool.tile([P, NG], f32, tag=f"bias{i}", name=f"bias{i}")
                nc.sync.dma_start(bt[:], bias_qk[i][:, :])
                bias_t.append(bt)
            bv_f = cpool.tile([1, DP], f32, tag="bvf", name="bvf")
            nc.sync.dma_start(bv_f[:], bv[:, :])
            bv16 = cpool.tile([1, DP], bf16, tag="bv16", name="bv16")
            nc.vector.tensor_copy(bv16[:], bv_f[:])
            ones_row = cpool.tile([1, P], bf16, tag="ones", name="ones_row")
            nc.gpsimd.memset(ones_row[:], 1.0)

            # ---- xbar transposes into SBUF (V weights first, x tok-major) ----
            wT = [[wT_pool.tile([P, DP], bf16, tag=f"wT{w}_{c}",
                                name=f"wT{w}_{c}")
                   for c in range(NHB)] for w in range(3)]
            for c in range(NHB):
                nc.sync.dma_start(wT[2][c][:], w_in[2][:, c * P:(c + 1) * P],
                                  transpose=True)
            xT = [xT_pool.tile([P, SEQ], bf16, tag=f"xT{c}", name=f"xT{c}")
                  for c in range(NHB)]
            for rc in range(4):
                r = slice(rc * 512, (rc + 1) * 512)
                for c in range(NHB):
                    nc.sync.dma_start(xT[c][:, r], x16[r, c * P:(c + 1) * P],
                                      transpose=True)
            for w in range(2):
                for c in range(NHB):
                    nc.sync.dma_start(wT[w][c][:],
                                      w_in[w][:, c * P:(c + 1) * P],
                                      transpose=True)

            # ---- V projection (natural layout) + V' assembly ----
            vp = vp_pool.tile([P, NT * NH * VW], bf16, tag="vp", name="vp")
            nc.gpsimd.memset(vp[:], 1.0)
            for tt in range(NT):
                ps = ps_proj.tile([P, DP], f32, tag="proj", name="ps_v")
                for p in range(NHB):
                    nc.tensor.matmul(ps[:],
                                     xT[p][:, tt * P:(tt + 1) * P],
                                     wT[2][p][:],
                                     start=(p == 0), stop=False)
                nc.tensor.matmul(ps[:], ones_row[:], bv16[:],
                                 start=False, stop=True)
                base = tt * NH * VW
                dst = vp[:, base:base + NH * VW]
                dst = dst.rearrange("p (h c) -> p h c", c=VW)[:, :, 0:HD]
                nc.vector.tensor_copy(dst, ps[:].rearrange("p (h c) -> p h c",
                                                           c=HD))

            # ---- Q^T / K^T projections (transposed layout) + attention ----
            qkT = [[qkT_pool.tile([P, SEQ], bf16, tag=f"qkT{w}_{g}",
                                  name=f"qkT{w}_{g}")
                    for g in range(NG)] for w in range(2)]
            for g in range(NG):
                gs = slice(g * P, (g + 1) * P)
                for w in range(2):
                    for tp in range(2):
                        pss = [ps_proj.tile([P, 512], f32, tag="proj",
                                            name="ps_qk")
                               for _ in range(2)]
                        for p in range(NHB):
                            for half in range(2):
                                tch = tp * 2 + half
                                nc.tensor.matmul(
                                    pss[half][:],
                                    wT[w][p][:, gs],
                                    xT[p][:, tch * 512:(tch + 1) * 512],
                                    start=(p == 0), stop=(p == NHB - 1))
                        for half in range(2):
                            tch = tp * 2 + half
                            nc.vector.tensor_scalar_add(
                                qkT[w][g][:, tch * 512:(tch + 1) * 512],
                                pss[half][:],
                                bias_t[w][:, g:g + 1])

                # ---- attention for the head pair of this group ----
                # head0 (partitions 0:64) and head1 (64:128) scores matmuls are
                # row-packed and run concurrently on the PE. head0's PV runs
                # inline; head1's exp tiles persist and its PV is a 2nd pass.
                h0, h1 = 2 * g, 2 * g + 1
                sl0, sl1 = slice(0, HD), slice(HD, P)
                for qb in range(2):
                    et1 = []
                    pvt0 = [ps_pv.tile([VW, 512], f32, tag="pv",
                                       name="pv0") for _ in range(2)]
                    for kt in range(NT):
                        ks = slice(kt * P, (kt + 1) * P)
                        sc0 = ps_sc.tile([P, 1024], f32, tag="sc", name="sc0")
                        sc1 = ps_sc.tile([P, 1024], f32, tag="sc", name="sc1")
                        for qh in range(2):
                            q0 = qb * 1024 + qh * 512
                            qs = slice(q0, q0 + 512)
                            os_ = slice(qh * 512, (qh + 1) * 512)
                            nc.tensor.matmul(sc0[:, os_], qkT[1][g][sl0, ks],
                                             qkT[0][g][sl0, qs],
                                             start=True, stop=True)
                            nc.tensor.matmul(sc1[:, os_], qkT[1][g][sl1, ks],
                                             qkT[0][g][sl1, qs],
                                             start=True, stop=True)
                        et0 = exp_pool.tile([P, 1024], bf16, tag="exp",
                                            name="et0")
                        nc.scalar.activation(et0[:], sc0[:], Exp,
                                             bias=mask_t[:, kt:kt + 1],
                                             scale=0.125)
                        et1k = exp1_pool.tile([P, 1024], bf16, tag="exp1",
                                              name="et1")
                        nc.scalar.activation(et1k[:], sc1[:], Exp,
                                             bias=mask_t[:, kt:kt + 1],
                                             scale=0.125)
                        et1.append(et1k)
                        vb0 = kt * NH * VW + h0 * VW
                        for qc in range(2):
                            nc.tensor.matmul(pvt0[qc][:],
                                             vp[:, vb0:vb0 + VW],
                                             et0[:, qc * 512:(qc + 1) * 512],
                                             start=(kt == 0),
                                             stop=(kt == NT - 1))
                    for qc in range(2):
                        ot = out_pool.tile([VW, 512], f32, tag="ot", name="ot")
                        nc.vector.tensor_copy(ot[:], pvt0[qc][:])
                        q0 = qb * 1024 + qc * 512
                        nc.sync.dma_start(out[h0][:, q0:q0 + 512], ot[:])
                    # second pass: head1's PV from the persisted exp tiles
                    pvt1 = [ps_pv.tile([VW, 512], f32, tag="pv",
                                       name="pv1") for _ in range(2)]
                    for kt in range(NT):
                        vb1 = kt * NH * VW + h1 * VW
                        for qc in range(2):
                            nc.tensor.matmul(pvt1[qc][:],
                                             vp[:, vb1:vb1 + VW],
                                             et1[kt][:, qc * 512:(qc + 1) * 512],
                                             start=(kt == 0),
                                             stop=(kt == NT - 1))
                    for qc in range(2):
                        ot = out_pool.tile([VW, 512], f32, tag="ot", name="ot")
                        nc.vector.tensor_copy(ot[:], pvt1[qc][:])
                        q0 = qb * 1024 + qc * 512
                        nc.sync.dma_start(out[h1][:, q0:q0 + 512], ot[:])

    nc.compile()
    return nc


def _get_nc():
    if "nc" not in _CACHE:
        _CACHE["nc"] = _build()
    return _CACHE["nc"]


def kernel(hidden_states, attention_mask, Wq, bq, Aq, Bq, Wk, bk, Ak, Bk,
           Wv, bv, Av, Bv):
    from concourse import bass_utils
    import ml_dtypes
    import os

    nc = _get_nc()
    bf = ml_dtypes.bfloat16

    hs = np.asarray(hidden_states, dtype=np.float32)
    am = np.asarray(attention_mask, dtype=np.float32)
    weff = {}
    for n, W, A, B in (("q", Wq, Aq, Bq), ("k", Wk, Ak, Bk), ("v", Wv, Av, Bv)):
        W = np.asarray(W, dtype=np.float32)
        A = np.asarray(A, dtype=np.float32)
        B = np.asarray(B, dtype=np.float32)
        weff[n] = (W + LORA_SCALING * (B @ A)).astype(bf)
    biases = {"q": np.asarray(bq, np.float32), "k": np.asarray(bk, np.float32),
              "v": np.asarray(bv, np.float32)}
    hs16 = hs.astype(bf)

    in_maps = []
    for c in range(N_CORES):
        b, hg = divmod(c, 2)
        rows = slice(hg * DP, (hg + 1) * DP)
        m = {
            "x16": np.ascontiguousarray(hs16[b]),
            "mask": np.ascontiguousarray(am[b, 0, 0].reshape(NT, P).T),
            "bv": biases["v"][rows].reshape(1, DP),
        }
        for n in ("q", "k", "v"):
            m[f"w{n}"] = np.ascontiguousarray(weff[n][rows])
        for n in ("q", "k"):
            m[f"b{n}"] = np.ascontiguousarray(
                biases[n][rows].reshape(NG, P).T)
        in_maps.append(m)

    trace = bool(int(os.environ.get("BASS_KERNEL_TRACE", "0")))
    res = bass_utils.run_bass_kernel_spmd(nc, in_maps,
                                          core_ids=list(range(N_CORES)),
                                          trace=trace)
    _CACHE["last_results"] = res

    output = np.empty((BS, SEQ, HID), dtype=np.float32)
    for c in range(N_CORES):
        b, hg = divmod(c, 2)
        r = res.results[c]["out"]                      # [NH, 65, SEQ]
        o = r[:, :HD, :] / r[:, HD:HD + 1, :]          # [NH, 64, SEQ]
        output[b, :, hg * DP:(hg + 1) * DP] = (
            o.transpose(2, 0, 1).reshape(SEQ, DP))
    return output
